# revision 18
# baseline (speedup 1.0000x reference)
"""DRAGConv (GATv2-style edge-softmax message passing) on 8 trn2 cores.

Strategy (dst-sorted, edge-gather, mask-matmul aggregation):
  - Host: fold |attn| into Wsrc/Wdst (leaky-relu sign trick), sort edges by
    dst, partition nodes into 8 contiguous ranges with ~equal edge counts,
    pack per-core tiles (<=128 dst nodes, <=1024 lo-src edges, <=1024
    hi-src edges), build per-tile gather indices + one-hot masks.
  - Device phase A: project el'|v tables for ALL nodes (replicated), er'
    for own nodes only (kept in SBUF).
  - Device phase B: per tile: dma_gather [el'|v] rows for 2048 edge slots,
    per 128-slot chunk: psum_u = MT_c @ er_tile + I @ el_rows;
    L = LeakyRelu(psum_u); logits = reduce(L * sigma) per head;
    ex = exp(logits); msgs = [v * ex | ex]; psum_agg += M_c @ msgs.
    Tail: out = psum_agg[:, :256] / denom per head, indirect-scatter to
    the core-local output rows.

Math: leaky_relu(x, a) . attn == sum_d sigma_d * leaky_relu(u_d, a) where
u = x * |attn| (fold into W) and sigma = sign(attn), because
LR(x)*w = sign(w) * LR(x*|w|).
"""
import sys

try:  # the runtime image ships concourse on the nix path
    import concourse.bass  # noqa: F401
except ImportError:  # fall back to the repo checkout
    sys.path.insert(0, "/opt/trn_rl_repo")

import numpy as np
import ml_dtypes

H, D = 4, 64
HD = H * D
ALPHA = 0.2
NCORES = 8
SLOTS = 2048          # edge slots per tile (16 chunks x 128)
NCHUNK = 16
LO_CHUNKS = 8         # chunks 0..7 from lo table, 8..15 from hi table
LO_CAP = LO_CHUNKS * 128
HI_CAP = (NCHUNK - LO_CHUNKS) * 128
TILE_NODES = 128


def preprocess(feat, Wsrc, bsrc, Wdst, bdst, Wv, bv, attn, src, dst):
    N, IN = feat.shape
    E = src.shape[0]
    src = np.asarray(src).astype(np.int64)
    dst = np.asarray(dst).astype(np.int64)

    attn_f = np.asarray(attn).reshape(-1).astype(np.float32)
    sigma = np.where(attn_f >= 0, np.float32(1.0), np.float32(-1.0))
    aabs = np.abs(attn_f)

    # fold |attn| into the src/dst projections (scale output rows)
    Wsrc_s = (np.asarray(Wsrc) * aabs[:, None]).astype(np.float32)
    Wdst_s = (np.asarray(Wdst) * aabs[:, None]).astype(np.float32)
    bsrc_s = (np.asarray(bsrc) * aabs).astype(np.float32)
    bdst_s = (np.asarray(bdst) * aabs).astype(np.float32)
    Wv_f = np.asarray(Wv).astype(np.float32)
    bv_f = np.asarray(bv).astype(np.float32)

    use_bias = bool(np.abs(bsrc_s).max() > 0 or np.abs(bdst_s).max() > 0
                    or np.abs(bv_f).max() > 0)

    # node-tile padding for the projection table (lo/hi split at 196*128)
    PN = ((N + 127) // 128) * 128
    NSPLIT = (PN // 2 // 128) * 128  # multiple of 128, < 32768
    assert NSPLIT < 32768 and PN - NSPLIT < 32768

    # dst-sorted edges
    order = np.argsort(dst, kind="stable")
    src_s = src[order]
    dst_s = dst[order]

    # core boundaries: contiguous node ranges with ~equal edge counts
    deg = np.bincount(dst_s, minlength=N)
    cume = np.concatenate([[0], np.cumsum(deg)])  # cume[n] = edges before node n
    node_b = [0]
    for c in range(1, NCORES):
        node_b.append(int(np.searchsorted(cume, c * E / NCORES)))
    node_b.append(N)

    # per-core tile packing (vectorized: prefix sums of lo/hi degree)
    lo_deg = np.bincount(dst_s[src_s < NSPLIT], minlength=N)
    cum_lo = np.concatenate([[0], np.cumsum(lo_deg)])
    cum_hi = cume - cum_lo
    cores = []
    for c in range(NCORES):
        nlo_n, nhi_n = node_b[c], node_b[c + 1]
        tiles = []  # each: (node_start, node_count, edge_start, edge_count)
        n = nlo_n
        while n < nhi_n:
            n0 = n
            n = min(
                n0 + TILE_NODES, nhi_n,
                int(np.searchsorted(cum_lo, cum_lo[n0] + LO_CAP, side="right")) - 1,
                int(np.searchsorted(cum_hi, cum_hi[n0] + HI_CAP, side="right")) - 1,
            )
            assert n > n0, f"node {n0} degree exceeds tile caps"
            tiles.append((n0, n - n0, int(cume[n0]), int(cume[n] - cume[n0])))
        cores.append(tiles)

    T = max(len(t) for t in cores)
    NL = max(node_b[c + 1] - node_b[c] for c in range(NCORES))
    NL = ((NL + 127) // 128) * 128

    # per-core data arrays
    f8 = ml_dtypes.float8_e4m3
    per_core = []
    for c in range(NCORES):
        tiles = cores[c]
        gidx = np.zeros((T, 128, 128), np.int16)     # dma_gather index layout (16-wrap tiled 8x)
        masks = np.zeros((T, 128, NCHUNK, 128), f8)   # lhsT for aggregate: [slot][chunk][node]
        maskT = np.zeros((T, 128, NCHUNK, 128), f8)   # lhsT for expand: [node][chunk][slot]
        outidx = np.full((T, 128, 1), NL, np.int32)   # local out row per node slot (NL => dropped)
        ernode = np.zeros((T * 128,), np.int64)       # global node per er slot
        ervalid = np.zeros((T * 128,), bool)

        for t, (n0, nn, e0, ne) in enumerate(tiles):
            es = src_s[e0:e0 + ne]
            ed = dst_s[e0:e0 + ne]
            lo_m = es < NSPLIT
            lo_src = es[lo_m]
            hi_src = es[~lo_m] - NSPLIT
            lsl = np.arange(lo_src.shape[0])          # slots 0..
            hsl = HI_CAP * 0 + LO_CAP + np.arange(hi_src.shape[0])
            slot = np.empty(ne, np.int64)
            slot[lo_m] = lsl
            slot[~lo_m] = hsl
            local_node = ed - n0                      # 0..nn-1

            # gather indices (seq position i lives at [i%16, i//16] of [16,128])
            seq = np.zeros(SLOTS, np.int16)
            seq[lsl] = lo_src.astype(np.int16)
            seq[LO_CAP + np.arange(hi_src.shape[0])] = hi_src.astype(np.int16)
            gidx[t] = np.tile(seq.reshape(16, 128, order="F"), (8, 1))  # [i%16, i//16], replicated per Q7 core

            ch = slot // 128
            sl = slot % 128
            masks[t, sl, ch, local_node] = 1.0
            maskT[t, local_node, ch, sl] = 1.0
            outidx[t, :nn, 0] = (n0 - node_b[c]) + np.arange(nn)
            ernode[t * 128: t * 128 + nn] = n0 + np.arange(nn)
            ervalid[t * 128: t * 128 + nn] = True

        # er projection source: feat rows in tile-slot order, transposed fp16
        fown = np.zeros((T * 128, IN), np.float32)
        fown[ervalid] = feat[ernode[ervalid]]
        fownT16 = np.ascontiguousarray(fown.T).astype(np.float16)

        per_core.append(dict(gidx=gidx, masks=masks, maskT=maskT,
                             outidx=outidx, fownT16=fownT16))

    featP = np.zeros((PN, IN), np.float32)
    featP[:N] = feat
    featT16 = np.ascontiguousarray(featP.T).astype(np.float16)

    WsWv = np.concatenate([Wsrc_s.T, Wv_f.T], axis=1).astype(np.float16)  # [IN, 512]
    WdT16 = np.ascontiguousarray(Wdst_s.T).astype(np.float16)             # [IN, 256]
    bias_sv = np.concatenate([bsrc_s, bv_f]).reshape(1, 512).astype(np.float16)
    bias_d = bdst_s.reshape(1, 256).astype(np.float16)
    sigma_rep = np.tile(np.repeat(sigma.reshape(1, HD), 128, axis=0).astype(np.float16), (1, 4))

    meta = dict(N=N, E=E, IN=IN, PN=PN, NSPLIT=NSPLIT, T=T, NL=NL,
                node_b=node_b, use_bias=use_bias)
    shared = dict(featT16=featT16, WsWv=WsWv, WdT16=WdT16,
                  bias_sv=bias_sv, bias_d=bias_d, sigma_rep=sigma_rep)
    return meta, shared, per_core


# ---------------------------------------------------------------------------
# device program
# ---------------------------------------------------------------------------

def build_program(meta, mask_fp8=False, sgrp=4):
    import concourse.bass as bass
    import concourse.tile as tile
    from concourse import bacc, mybir

    FP16 = mybir.dt.float16
    FP32 = mybir.dt.float32
    FP8 = mybir.dt.float8e4
    I16 = mybir.dt.int16
    I32 = mybir.dt.int32
    ts = bass.ts

    IN = meta["IN"]
    PN, NSPLIT, T, NL = meta["PN"], meta["NSPLIT"], meta["T"], meta["NL"]
    use_bias = meta["use_bias"]
    PT = PN // 128
    PT_LO = NSPLIT // 128
    HI_ROWS = PN - NSPLIT
    MDT = FP8 if mask_fp8 else FP16
    NG = NCHUNK // sgrp  # chunk groups per tile

    nc = bacc.Bacc("TRN2", target_bir_lowering=False, debug=False, num_devices=1,
                   num_swdge_queues=4)

    featT = nc.dram_tensor("featT", [2, 128, PN], FP16, kind="ExternalInput")
    fownT = nc.dram_tensor("fownT", [2, 128, T * 128], FP16, kind="ExternalInput")
    wsv_d = nc.dram_tensor("wsv", [IN, 512], FP16, kind="ExternalInput")
    wd_d = nc.dram_tensor("wd", [IN, 256], FP16, kind="ExternalInput")
    bsv_d = nc.dram_tensor("bsv", [1, 512], FP16, kind="ExternalInput")
    bd_d = nc.dram_tensor("bd", [1, 256], FP16, kind="ExternalInput")
    sig_d = nc.dram_tensor("sig", [128, 1024], FP16, kind="ExternalInput")
    gidx_d = nc.dram_tensor("gidx", [T, 128, 128], I16, kind="ExternalInput")
    mk_d = nc.dram_tensor("mk", [T, 128, NCHUNK, 128], MDT, kind="ExternalInput")
    mkT_d = nc.dram_tensor("mkT", [T, 128, NCHUNK, 128], MDT, kind="ExternalInput")
    ident_d = nc.dram_tensor("ident", [128, 128], FP16, kind="ExternalInput")
    out_d = nc.dram_tensor("out", [T, 128, 256], FP32, kind="ExternalOutput")

    tab_lo = nc.dram_tensor("tab_lo", [NSPLIT, 512], FP16, kind="Internal")
    tab_hi = nc.dram_tensor("tab_hi", [HI_ROWS, 512], FP16, kind="Internal")

    with tile.TileContext(nc) as tc:
        with (
            tc.tile_pool(name="const", bufs=1) as constp,
            tc.tile_pool(name="erall", bufs=1) as erp,
            tc.tile_pool(name="ftin", bufs=3) as ftp,
            tc.tile_pool(name="gout", bufs=3) as gop,
            tc.tile_pool(name="psU", bufs=3, space="PSUM") as psU,
            tc.tile_pool(name="psA", bufs=2, space="PSUM") as psA,
            tc.tile_pool(name="gath", bufs=4) as gat,
            tc.tile_pool(name="mask", bufs=4) as mkp,
            tc.tile_pool(name="idxp", bufs=2) as idxp,
            tc.tile_pool(name="lwork", bufs=4) as lwp,
            tc.tile_pool(name="msgs", bufs=4) as msp,
            tc.tile_pool(name="small", bufs=4) as smp,
            tc.tile_pool(name="outp", bufs=2) as outp,
        ):
            # constants
            wsv0 = constp.tile([128, 512], FP16, tag="wsv0")
            nc.sync.dma_start(wsv0[:], wsv_d.ap()[0:128, :])
            wsv1 = constp.tile([128, 512], FP16, tag="wsv1")
            nc.sync.dma_start(wsv1[:], wsv_d.ap()[128:256, :])
            wd0 = constp.tile([128, 256], FP16, tag="wd0")
            nc.sync.dma_start(wd0[:], wd_d.ap()[0:128, :])
            wd1 = constp.tile([128, 256], FP16, tag="wd1")
            nc.sync.dma_start(wd1[:], wd_d.ap()[128:256, :])
            sig = constp.tile([128, 1024], FP16, tag="sig")
            nc.sync.dma_start(sig[:], sig_d.ap()[:])
            ident = constp.tile([128, 128], FP16, tag="ident")
            nc.sync.dma_start(ident[:], ident_d.ap()[:])
            alpha = constp.tile([128, 1], FP32, tag="alpha")
            nc.vector.memset(alpha[:], ALPHA)
            if use_bias:
                bsv = constp.tile([1, 512], FP16, tag="bsv")
                nc.sync.dma_start(bsv[:], bsv_d.ap()[:])
                bd = constp.tile([1, 256], FP16, tag="bd")
                nc.sync.dma_start(bd[:], bd_d.ap()[:])
                ones = constp.tile([1, 128], FP16, tag="ones")
                nc.vector.memset(ones[:], 1.0)

            er_all = erp.tile([128, T, 256], FP16, tag="er_all")

            GRP = 8  # node tiles per featT load group

            # ---- phase A1: el'|v tables for all nodes (replicated) ----
            # Emitted as lo-table groups, then er' (A2), then hi-table groups:
            # tab_lo + er_all complete early so phase-B lo gathers and compute
            # overlap the hi-table tail.
            def a1_group(g):
                npt = min(GRP, PT - g * GRP)
                w = npt * 128
                f0 = ftp.tile([128, GRP * 128], FP16, tag="f0")
                nc.scalar.dma_start(f0[:, 0:w], featT.ap()[0, :, g * GRP * 128:g * GRP * 128 + w])
                f1 = ftp.tile([128, GRP * 128], FP16, tag="f1")
                nc.scalar.dma_start(f1[:, 0:w], featT.ap()[1, :, g * GRP * 128:g * GRP * 128 + w])
                g16 = gop.tile([128, GRP, 512], FP16, tag="g16")
                for j in range(npt):
                    psgt = psU.tile([128, sgrp, 256], FP32, tag="psu")
                    psg = psgt.rearrange("p a b -> p (a b)")[:, 0:512]
                    nc.tensor.matmul(psg, f0[:, ts(j, 128)], wsv0[:], start=True, stop=False)
                    nc.tensor.matmul(psg, f1[:, ts(j, 128)], wsv1[:],
                                     start=False, stop=not use_bias)
                    if use_bias:
                        nc.tensor.matmul(psg, ones[:], bsv[:], start=False, stop=True)
                    # PSUM->fp16 cast: alternate engines so neither becomes
                    # the phase-A bottleneck
                    if j % 2 == 0:
                        nc.vector.tensor_copy(g16[:, j, :], psg)
                    else:
                        nc.scalar.activation(g16[:, j, :], psg,
                                             mybir.ActivationFunctionType.Copy)
                # one batched write per group; rows pt*128..(pt+npt)*128 of the
                # combined table, split at the lo/hi boundary if it lands inside
                p0 = g * GRP
                segs = []
                if p0 < PT_LO:
                    e = min(p0 + npt, PT_LO)
                    segs.append((0, e - p0, tab_lo, p0))
                if p0 + npt > PT_LO:
                    s = max(p0, PT_LO)
                    segs.append((s - p0, p0 + npt - s, tab_hi, s - PT_LO))
                for (joff, cnt, tab, rt) in segs:
                    nc.sync.dma_start(
                        tab.ap()[rt * 128:(rt + cnt) * 128, :].rearrange(
                            "(a p) w -> p a w", p=128),
                        g16[:, joff:joff + cnt, :])

            n_groups = (PT + GRP - 1) // GRP
            ng_lo = min((PT_LO + GRP - 1) // GRP, n_groups)
            for g in range(ng_lo):
                a1_group(g)

            # ---- phase A2: er' for own nodes, kept in SBUF ----
            for g in range((T + GRP - 1) // GRP):
                npt = min(GRP, T - g * GRP)
                w = npt * 128
                f0 = ftp.tile([128, GRP * 128], FP16, tag="f0")
                nc.scalar.dma_start(f0[:, 0:w], fownT.ap()[0, :, g * GRP * 128:g * GRP * 128 + w])
                f1 = ftp.tile([128, GRP * 128], FP16, tag="f1")
                nc.scalar.dma_start(f1[:, 0:w], fownT.ap()[1, :, g * GRP * 128:g * GRP * 128 + w])
                for j in range(npt):
                    t = g * GRP + j
                    pset = psU.tile([128, sgrp, 256], FP32, tag="psu")
                    pse = pset.rearrange("p a b -> p (a b)")
                    nc.tensor.matmul(pse[:, 0:256], f0[:, ts(j, 128)], wd0[:], start=True, stop=False)
                    nc.tensor.matmul(pse[:, 0:256], f1[:, ts(j, 128)], wd1[:],
                                     start=False, stop=not use_bias)
                    if use_bias:
                        nc.tensor.matmul(pse[:, 0:256], ones[:], bd[:], start=False, stop=True)
                    nc.scalar.activation(er_all[:, t, :], pse[:, 0:256],
                                         mybir.ActivationFunctionType.Copy)

            # ---- phase A1 (cont): hi-table groups ----
            for g in range(ng_lo, n_groups):
                a1_group(g)

            # ---- phase B ----
            def epilogue(pagg, t):
                rec = smp.tile([128, 4], FP32, tag="rec")
                nc.vector.tensor_scalar(rec[:], pagg[:, 256:260], 1e-30, None,
                                        op0=mybir.AluOpType.add)
                nc.vector.reciprocal(rec[:], rec[:])
                outt = outp.tile([128, 256], FP32, tag="outt")
                nc.vector.tensor_tensor(
                    outt[:].rearrange("p (h d) -> p h d", h=4),
                    pagg[:, 0:256].rearrange("p (h d) -> p h d", h=4),
                    rec[:].unsqueeze(-1).broadcast_to([128, 4, 64]),
                    op=mybir.AluOpType.mult)
                nc.sync.dma_start(out_d.ap()[t], outt[:])

            pending = None
            for t in range(T):
                tidx = idxp.tile([128, 128], I16, tag="tidx")
                with tc.high_priority():
                    nc.sync.dma_start(tidx[:], gidx_d.ap()[t])
                tglo = gat.tile([128, LO_CHUNKS, 512], FP16, tag="tglo")
                nc.gpsimd.dma_gather(tglo[:], tab_lo.ap()[:],
                                     tidx[:, 0:64], LO_CAP, LO_CAP, 512,
                                     queue_num=(2 * t) % 4)
                tm = mkp.tile([128, NCHUNK, 128], FP16, tag="tm")
                tmT = mkp.tile([128, NCHUNK, 128], FP16, tag="tmT")
                with tc.high_priority():
                    nc.scalar.dma_start(tm[:], mk_d.ap()[t])
                    nc.scalar.dma_start(tmT[:], mkT_d.ap()[t])

                pagg = psA.tile([128, 260], FP32, tag="pagg")
                tghi = None
                for gi in range(NG):
                    if gi == 1 and pending is not None:
                        # previous tile's normalize/writeback, deferred so the
                        # engines never idle waiting on its last agg matmul
                        epilogue(*pending)
                        pending = None
                    if gi == LO_CHUNKS // sgrp:
                        # hi-table gather issued mid-tile: lo-chunk compute
                        # overlaps it (and, for early tiles, the hi-table
                        # projection tail)
                        tghi = gat.tile([128, NCHUNK - LO_CHUNKS, 512], FP16,
                                        tag="tghi")
                        nc.gpsimd.dma_gather(tghi[:], tab_hi.ap()[:],
                                             tidx[:, 64:128], HI_CAP, HI_CAP, 512,
                                             queue_num=(2 * t + 1) % 4)
                    psu = psU.tile([128, sgrp, 256], FP32, tag="psu")
                    for j in range(sgrp):
                        c = gi * sgrp + j
                        nc.tensor.matmul(psu[:, j, :], tmT[:, c, :], er_all[:, t, :],
                                         start=True, stop=False)
                        tgc = tglo[:, c, 0:256] if c < LO_CHUNKS else tghi[:, c - LO_CHUNKS, 0:256]
                        nc.tensor.matmul(psu[:, j, :], ident[:], tgc,
                                         start=False, stop=True)
                    lrl = lwp.tile([128, sgrp * 256], FP16, tag="lrl")
                    nc.scalar.activation(lrl[:], psu[:].rearrange("p a b -> p (a b)"),
                                         mybir.ActivationFunctionType.Prelu, alpha=alpha[:])
                    lsg = lwp.tile([128, sgrp * 256], FP16, tag="lsg")
                    nc.vector.tensor_tensor(lsg[:], lrl[:], sig[:, 0:sgrp * 256],
                                            op=mybir.AluOpType.mult)
                    red = smp.tile([128, sgrp * 4], FP32, tag="red")
                    nc.vector.tensor_reduce(red[:], lsg[:].rearrange("p (a h d) -> p (a h) d", h=4, d=64),
                                            axis=mybir.AxisListType.X,
                                            op=mybir.AluOpType.add)
                    msgs = msp.tile([128, sgrp, 260], FP16, tag="msgs")
                    nc.scalar.activation(msgs[:, :, 256:260],
                                         red[:].rearrange("p (a h) -> p a h", h=4),
                                         mybir.ActivationFunctionType.Exp)
                    nc.vector.tensor_tensor(
                        msgs[:, :, 0:256].rearrange("p a (h d) -> p a h d", h=4, d=64),
                        (tglo if gi * sgrp < LO_CHUNKS else tghi)[:,
                            gi * sgrp - (0 if gi * sgrp < LO_CHUNKS else LO_CHUNKS):
                            gi * sgrp - (0 if gi * sgrp < LO_CHUNKS else LO_CHUNKS) + sgrp,
                            256:512].rearrange("p a (h d) -> p a h d", h=4, d=64),
                        msgs[:, :, 256:260].unsqueeze(-1).broadcast_to([128, sgrp, 4, 64]),
                        op=mybir.AluOpType.mult)
                    for j in range(sgrp):
                        c = gi * sgrp + j
                        nc.tensor.matmul(pagg[:], tm[:, c, :], msgs[:, j, :],
                                         start=(c == 0), stop=(c == NCHUNK - 1))

                pending = (pagg, t)
            epilogue(*pending)

    nc.compile()
    return nc


# ---------------------------------------------------------------------------
# kernel entry point
# ---------------------------------------------------------------------------

TRACE = False
LAST_RESULTS = None


def _ntff_hook_shim():
    """Register the axon NTFF profile hook if the antenv shim is missing."""
    import types
    try:
        from antenv.axon_hooks import get_axon_ntff_profile_hook  # noqa: F401
        return
    except ImportError:
        pass
    try:
        if '/root/.axon_site' not in sys.path:
            sys.path.insert(0, '/root/.axon_site')
        from trn_agent_boot.trn_boot import _ntff_profile_via_ctypes
        hook = _ntff_profile_via_ctypes('/opt/axon/libaxon_pjrt.so')
        mod = types.ModuleType('antenv.axon_hooks')
        mod.get_axon_ntff_profile_hook = lambda: hook
        sys.modules['antenv.axon_hooks'] = mod
    except Exception:
        pass


def _make_in_maps(meta, shared, per_core):
    T, PN = meta["T"], meta["PN"]
    ident = np.eye(128, dtype=np.float16)
    maps = []
    for pc in per_core:
        maps.append(dict(
            featT=shared["featT16"].reshape(2, 128, PN),
            fownT=pc["fownT16"].reshape(2, 128, T * 128),
            wsv=shared["WsWv"], wd=shared["WdT16"],
            bsv=shared["bias_sv"], bd=shared["bias_d"],
            sig=shared["sigma_rep"],
            gidx=pc["gidx"], ident=ident,
            mk=pc["masks"].astype(np.float16),
            mkT=pc["maskT"].astype(np.float16),
        ))
    return maps


def kernel(feat, Wsrc, bsrc, Wdst, bdst, Wv, bv, attn, src, dst):
    global LAST_RESULTS
    from concourse.bass_utils import run_bass_kernel_spmd

    feat = np.asarray(feat, dtype=np.float32)
    args = dict(feat=feat,
                Wsrc=np.asarray(Wsrc, np.float32), bsrc=np.asarray(bsrc, np.float32),
                Wdst=np.asarray(Wdst, np.float32), bdst=np.asarray(bdst, np.float32),
                Wv=np.asarray(Wv, np.float32), bv=np.asarray(bv, np.float32),
                attn=np.asarray(attn, np.float32),
                src=np.asarray(src), dst=np.asarray(dst))
    N = feat.shape[0]

    meta, shared, per_core = preprocess(**args)
    nc = build_program(meta, mask_fp8=False)
    in_maps = _make_in_maps(meta, shared, per_core)

    kwargs = {}
    if TRACE:
        _ntff_hook_shim()
        kwargs["trace"] = True
    res = run_bass_kernel_spmd(nc, in_maps, core_ids=list(range(NCORES)), **kwargs)
    LAST_RESULTS = res

    out = np.zeros((N, HD), np.float32)
    nb = meta["node_b"]
    for c, r in enumerate(res.results):
        staged = r["out"].reshape(meta["T"] * 128, HD)
        oix = per_core[c]["outidx"].reshape(-1)
        valid = oix < meta["NL"]
        out[nb[c] + oix[valid]] = staged[valid]
    return out.reshape(N, H, D)



# revision 28
# speedup vs baseline: 1.1079x; 1.1079x over previous
"""DRAGConv (GATv2-style edge-softmax message passing) on 8 trn2 cores.

Strategy (dst-sorted, edge-gather, mask-matmul aggregation):
  - Host: fold |attn| into Wsrc/Wdst (leaky-relu sign trick), sort edges by
    dst, partition nodes into 8 contiguous ranges with ~equal edge counts,
    pack per-core tiles (<=128 dst nodes, <=1024 lo-src edges, <=1024
    hi-src edges), build per-tile gather indices + one-hot masks.
  - Device phase A: project el'|v tables for ALL nodes (replicated), er'
    for own nodes only (kept in SBUF).
  - Device phase B: per tile: dma_gather [el'|v] rows for 2048 edge slots,
    per 128-slot chunk: psum_u = MT_c @ er_tile + I @ el_rows;
    L = LeakyRelu(psum_u); logits = reduce(L * sigma) per head;
    ex = exp(logits); msgs = [v * ex | ex]; psum_agg += M_c @ msgs.
    Tail: out = psum_agg[:, :256] / denom per head, indirect-scatter to
    the core-local output rows.

Math: leaky_relu(x, a) . attn == sum_d sigma_d * leaky_relu(u_d, a) where
u = x * |attn| (fold into W) and sigma = sign(attn), because
LR(x)*w = sign(w) * LR(x*|w|).
"""
import sys

try:  # the runtime image ships concourse on the nix path
    import concourse.bass  # noqa: F401
except ImportError:  # fall back to the repo checkout
    sys.path.insert(0, "/opt/trn_rl_repo")

import numpy as np
import ml_dtypes

H, D = 4, 64
HD = H * D
ALPHA = 0.2
NCORES = 8
SLOTS = 2048          # edge slots per tile (16 chunks x 128)
NCHUNK = 16
LO_CHUNKS = 8         # chunks 0..7 from lo table, 8..15 from hi table
LO_CAP = LO_CHUNKS * 128
HI_CAP = (NCHUNK - LO_CHUNKS) * 128
TILE_NODES = 128


def preprocess(feat, Wsrc, bsrc, Wdst, bdst, Wv, bv, attn, src, dst):
    N, IN = feat.shape
    E = src.shape[0]
    src = np.asarray(src).astype(np.int64)
    dst = np.asarray(dst).astype(np.int64)

    attn_f = np.asarray(attn).reshape(-1).astype(np.float32)
    sigma = np.where(attn_f >= 0, np.float32(1.0), np.float32(-1.0))
    aabs = np.abs(attn_f)

    # fold |attn| into the src/dst projections (scale output rows)
    Wsrc_s = (np.asarray(Wsrc) * aabs[:, None]).astype(np.float32)
    Wdst_s = (np.asarray(Wdst) * aabs[:, None]).astype(np.float32)
    bsrc_s = (np.asarray(bsrc) * aabs).astype(np.float32)
    bdst_s = (np.asarray(bdst) * aabs).astype(np.float32)
    Wv_f = np.asarray(Wv).astype(np.float32)
    bv_f = np.asarray(bv).astype(np.float32)

    use_bias = bool(np.abs(bsrc_s).max() > 0 or np.abs(bdst_s).max() > 0
                    or np.abs(bv_f).max() > 0)

    # node-tile padding for the projection table (lo/hi split at 196*128)
    PN = ((N + 127) // 128) * 128
    NSPLIT = (PN // 2 // 128) * 128  # multiple of 128, < 32768
    assert NSPLIT < 32768 and PN - NSPLIT < 32768

    # dst-sorted edges
    order = np.argsort(dst, kind="stable")
    src_s = src[order]
    dst_s = dst[order]

    # core boundaries: contiguous node ranges with ~equal edge counts
    deg = np.bincount(dst_s, minlength=N)
    cume = np.concatenate([[0], np.cumsum(deg)])  # cume[n] = edges before node n
    node_b = [0]
    for c in range(1, NCORES):
        node_b.append(int(np.searchsorted(cume, c * E / NCORES)))
    node_b.append(N)

    # per-core tile packing (vectorized: prefix sums of lo/hi degree)
    lo_deg = np.bincount(dst_s[src_s < NSPLIT], minlength=N)
    cum_lo = np.concatenate([[0], np.cumsum(lo_deg)])
    cum_hi = cume - cum_lo
    cores = []
    for c in range(NCORES):
        nlo_n, nhi_n = node_b[c], node_b[c + 1]
        tiles = []  # each: (node_start, node_count, edge_start, edge_count)
        n = nlo_n
        while n < nhi_n:
            n0 = n
            n = min(
                n0 + TILE_NODES, nhi_n,
                int(np.searchsorted(cum_lo, cum_lo[n0] + LO_CAP, side="right")) - 1,
                int(np.searchsorted(cum_hi, cum_hi[n0] + HI_CAP, side="right")) - 1,
            )
            assert n > n0, f"node {n0} degree exceeds tile caps"
            tiles.append((n0, n - n0, int(cume[n0]), int(cume[n] - cume[n0])))
        cores.append(tiles)

    T = max(len(t) for t in cores)
    NL = max(node_b[c + 1] - node_b[c] for c in range(NCORES))
    NL = ((NL + 127) // 128) * 128

    # per-core data arrays
    f8 = ml_dtypes.float8_e4m3
    per_core = []
    for c in range(NCORES):
        tiles = cores[c]
        gidx = np.zeros((T, 128, 128), np.int16)     # dma_gather index layout (16-wrap tiled 8x)
        masks = np.zeros((T, 128, NCHUNK, 128), f8)   # lhsT for aggregate: [slot][chunk][node]
        maskT = np.zeros((T, 128, NCHUNK, 128), f8)   # lhsT for expand: [node][chunk][slot]
        outidx = np.full((T, 128, 1), NL, np.int32)   # local out row per node slot (NL => dropped)
        ernode = np.zeros((T * 128,), np.int64)       # global node per er slot
        ervalid = np.zeros((T * 128,), bool)

        for t, (n0, nn, e0, ne) in enumerate(tiles):
            es = src_s[e0:e0 + ne]
            ed = dst_s[e0:e0 + ne]
            lo_m = es < NSPLIT
            lo_src = es[lo_m]
            hi_src = es[~lo_m] - NSPLIT
            lsl = np.arange(lo_src.shape[0])          # slots 0..
            hsl = HI_CAP * 0 + LO_CAP + np.arange(hi_src.shape[0])
            slot = np.empty(ne, np.int64)
            slot[lo_m] = lsl
            slot[~lo_m] = hsl
            local_node = ed - n0                      # 0..nn-1

            # gather indices (seq position i lives at [i%16, i//16] of [16,128])
            seq = np.zeros(SLOTS, np.int16)
            seq[lsl] = lo_src.astype(np.int16)
            seq[LO_CAP + np.arange(hi_src.shape[0])] = hi_src.astype(np.int16)
            gidx[t] = np.tile(seq.reshape(16, 128, order="F"), (8, 1))  # [i%16, i//16], replicated per Q7 core

            ch = slot // 128
            sl = slot % 128
            masks[t, sl, ch, local_node] = 1.0
            maskT[t, local_node, ch, sl] = 1.0
            outidx[t, :nn, 0] = (n0 - node_b[c]) + np.arange(nn)
            ernode[t * 128: t * 128 + nn] = n0 + np.arange(nn)
            ervalid[t * 128: t * 128 + nn] = True

        # er projection source: feat rows in tile-slot order, transposed fp16
        fown = np.zeros((T * 128, IN), np.float32)
        fown[ervalid] = feat[ernode[ervalid]]
        fownT16 = np.ascontiguousarray(fown.T).astype(np.float16)

        per_core.append(dict(gidx=gidx, masks=masks, maskT=maskT,
                             outidx=outidx, fownT16=fownT16))

    featP = np.zeros((PN, IN), np.float32)
    featP[:N] = feat
    featT16 = np.ascontiguousarray(featP.T).astype(np.float16)

    WsWv = np.concatenate([Wsrc_s.T, Wv_f.T], axis=1).astype(np.float16)  # [IN, 512]
    WdT16 = np.ascontiguousarray(Wdst_s.T).astype(np.float16)             # [IN, 256]
    bias_sv = np.concatenate([bsrc_s, bv_f]).reshape(1, 512).astype(np.float16)
    bias_d = bdst_s.reshape(1, 256).astype(np.float16)
    sigma_rep = np.tile(np.repeat(sigma.reshape(1, HD), 128, axis=0).astype(np.float16), (1, 4))

    meta = dict(N=N, E=E, IN=IN, PN=PN, NSPLIT=NSPLIT, T=T, NL=NL,
                node_b=node_b, use_bias=use_bias)
    shared = dict(featT16=featT16, WsWv=WsWv, WdT16=WdT16,
                  bias_sv=bias_sv, bias_d=bias_d, sigma_rep=sigma_rep)
    return meta, shared, per_core


# ---------------------------------------------------------------------------
# device program
# ---------------------------------------------------------------------------

def build_program(meta, mask_fp8=False, sgrp=4):
    import concourse.bass as bass
    import concourse.tile as tile
    from concourse import bacc, mybir

    FP16 = mybir.dt.float16
    FP32 = mybir.dt.float32
    FP8 = mybir.dt.float8e4
    I16 = mybir.dt.int16
    I32 = mybir.dt.int32
    ts = bass.ts

    IN = meta["IN"]
    PN, NSPLIT, T, NL = meta["PN"], meta["NSPLIT"], meta["T"], meta["NL"]
    use_bias = meta["use_bias"]
    PT = PN // 128
    PT_LO = NSPLIT // 128
    HI_ROWS = PN - NSPLIT
    MDT = FP8 if mask_fp8 else FP16
    NG = NCHUNK // sgrp  # chunk groups per tile

    nc = bacc.Bacc("TRN2", target_bir_lowering=False, debug=False, num_devices=1,
                   num_swdge_queues=4)

    featT = nc.dram_tensor("featT", [2, 128, PN], FP16, kind="ExternalInput")
    fownT = nc.dram_tensor("fownT", [2, 128, T * 128], FP16, kind="ExternalInput")
    wsv_d = nc.dram_tensor("wsv", [IN, 512], FP16, kind="ExternalInput")
    wd_d = nc.dram_tensor("wd", [IN, 256], FP16, kind="ExternalInput")
    bsv_d = nc.dram_tensor("bsv", [1, 512], FP16, kind="ExternalInput")
    bd_d = nc.dram_tensor("bd", [1, 256], FP16, kind="ExternalInput")
    sig_d = nc.dram_tensor("sig", [128, 1024], FP16, kind="ExternalInput")
    gidx_d = nc.dram_tensor("gidx", [T, 128, 128], I16, kind="ExternalInput")
    mk_d = nc.dram_tensor("mk", [T, 128, NCHUNK, 128], MDT, kind="ExternalInput")
    mkT_d = nc.dram_tensor("mkT", [T, 128, NCHUNK, 128], MDT, kind="ExternalInput")
    ident_d = nc.dram_tensor("ident", [128, 128], FP8, kind="ExternalInput")
    out_d = nc.dram_tensor("out", [T, 128, 256], FP32, kind="ExternalOutput")

    # table row layout: 256 x fp8 el' | 512 bytes (256 x fp16, bitcast) v.
    # fp8 el costs ~3% relative noise on the pre-activation attention input
    # (well inside tolerance) and cuts table write + gather bytes by 25%.
    tab_lo = nc.dram_tensor("tab_lo", [NSPLIT, 768], FP8, kind="Internal")
    tab_hi = nc.dram_tensor("tab_hi", [HI_ROWS, 768], FP8, kind="Internal")

    with tile.TileContext(nc) as tc:
        with (
            tc.tile_pool(name="const", bufs=1) as constp,
            tc.tile_pool(name="erall", bufs=1) as erp,
            tc.tile_pool(name="ftin", bufs=3) as ftp,
            tc.tile_pool(name="gout", bufs=3) as gop,
            tc.tile_pool(name="psU", bufs=3, space="PSUM") as psU,
            tc.tile_pool(name="psA", bufs=2, space="PSUM") as psA,
            tc.tile_pool(name="gath", bufs=4) as gat,
            tc.tile_pool(name="mask", bufs=4) as mkp,
            tc.tile_pool(name="idxp", bufs=2) as idxp,
            tc.tile_pool(name="lwork", bufs=4) as lwp,
            tc.tile_pool(name="msgs", bufs=4) as msp,
            tc.tile_pool(name="small", bufs=4) as smp,
            tc.tile_pool(name="outp", bufs=2) as outp,
        ):
            # constants
            wsv0 = constp.tile([128, 512], FP16, tag="wsv0")
            nc.sync.dma_start(wsv0[:], wsv_d.ap()[0:128, :])
            wsv1 = constp.tile([128, 512], FP16, tag="wsv1")
            nc.sync.dma_start(wsv1[:], wsv_d.ap()[128:256, :])
            wd0 = constp.tile([128, 256], FP16, tag="wd0")
            nc.sync.dma_start(wd0[:], wd_d.ap()[0:128, :])
            wd1 = constp.tile([128, 256], FP16, tag="wd1")
            nc.sync.dma_start(wd1[:], wd_d.ap()[128:256, :])
            sig = constp.tile([128, 1024], FP16, tag="sig")
            nc.sync.dma_start(sig[:], sig_d.ap()[:])
            ident = constp.tile([128, 128], FP8, tag="ident")
            nc.sync.dma_start(ident[:], ident_d.ap()[:])
            alpha = constp.tile([128, 1], FP32, tag="alpha")
            nc.vector.memset(alpha[:], ALPHA)
            if use_bias:
                bsv = constp.tile([1, 512], FP16, tag="bsv")
                nc.sync.dma_start(bsv[:], bsv_d.ap()[:])
                bd = constp.tile([1, 256], FP16, tag="bd")
                nc.sync.dma_start(bd[:], bd_d.ap()[:])
                ones = constp.tile([1, 128], FP16, tag="ones")
                nc.vector.memset(ones[:], 1.0)

            er_all = erp.tile([128, T, 256], FP16, tag="er_all")

            GRP = 8  # node tiles per featT load group

            # ---- phase A1: el'|v tables for all nodes (replicated) ----
            # Emitted as lo-table groups, then er' (A2), then hi-table groups:
            # tab_lo + er_all complete early so phase-B lo gathers and compute
            # overlap the hi-table tail.
            def a1_group(g):
                npt = min(GRP, PT - g * GRP)
                w = npt * 128
                f0 = ftp.tile([128, GRP * 128], FP16, tag="f0")
                nc.scalar.dma_start(f0[:, 0:w], featT.ap()[0, :, g * GRP * 128:g * GRP * 128 + w])
                f1 = ftp.tile([128, GRP * 128], FP16, tag="f1")
                nc.scalar.dma_start(f1[:, 0:w], featT.ap()[1, :, g * GRP * 128:g * GRP * 128 + w])
                g16 = gop.tile([128, GRP, 768], FP8, tag="g16")
                for j in range(npt):
                    psgt = psU.tile([128, sgrp, 256], FP32, tag="psu")
                    psg = psgt.rearrange("p a b -> p (a b)")[:, 0:512]
                    nc.tensor.matmul(psg, f0[:, ts(j, 128)], wsv0[:], start=True, stop=False)
                    nc.tensor.matmul(psg, f1[:, ts(j, 128)], wsv1[:],
                                     start=False, stop=not use_bias)
                    if use_bias:
                        nc.tensor.matmul(psg, ones[:], bsv[:], start=False, stop=True)
                    # PSUM casts (el -> fp8, v -> fp16 via bitcast view):
                    # alternate engines so neither becomes the phase-A pacer
                    eng = nc.vector if j % 2 == 0 else nc.scalar
                    if j % 2 == 0:
                        eng.tensor_copy(g16[:, j, 0:256], psg[:, 0:256])
                        eng.tensor_copy(g16[:, j, 256:768].bitcast(FP16),
                                        psg[:, 256:512])
                    else:
                        eng.activation(g16[:, j, 0:256], psg[:, 0:256],
                                       mybir.ActivationFunctionType.Copy)
                        eng.activation(g16[:, j, 256:768].bitcast(FP16),
                                       psg[:, 256:512],
                                       mybir.ActivationFunctionType.Copy)
                # one batched write per group; rows pt*128..(pt+npt)*128 of the
                # combined table, split at the lo/hi boundary if it lands inside
                p0 = g * GRP
                segs = []
                if p0 < PT_LO:
                    e = min(p0 + npt, PT_LO)
                    segs.append((0, e - p0, tab_lo, p0))
                if p0 + npt > PT_LO:
                    s = max(p0, PT_LO)
                    segs.append((s - p0, p0 + npt - s, tab_hi, s - PT_LO))
                for (joff, cnt, tab, rt) in segs:
                    nc.sync.dma_start(
                        tab.ap()[rt * 128:(rt + cnt) * 128, :].rearrange(
                            "(a p) w -> p a w", p=128),
                        g16[:, joff:joff + cnt, :])

            n_groups = (PT + GRP - 1) // GRP
            ng_lo = min((PT_LO + GRP - 1) // GRP, n_groups)
            for g in range(ng_lo):
                a1_group(g)

            # ---- phase A2: er' for own nodes, kept in SBUF ----
            for g in range((T + GRP - 1) // GRP):
                npt = min(GRP, T - g * GRP)
                w = npt * 128
                f0 = ftp.tile([128, GRP * 128], FP16, tag="f0")
                nc.scalar.dma_start(f0[:, 0:w], fownT.ap()[0, :, g * GRP * 128:g * GRP * 128 + w])
                f1 = ftp.tile([128, GRP * 128], FP16, tag="f1")
                nc.scalar.dma_start(f1[:, 0:w], fownT.ap()[1, :, g * GRP * 128:g * GRP * 128 + w])
                for j in range(npt):
                    t = g * GRP + j
                    pset = psU.tile([128, sgrp, 256], FP32, tag="psu")
                    pse = pset.rearrange("p a b -> p (a b)")
                    nc.tensor.matmul(pse[:, 0:256], f0[:, ts(j, 128)], wd0[:], start=True, stop=False)
                    nc.tensor.matmul(pse[:, 0:256], f1[:, ts(j, 128)], wd1[:],
                                     start=False, stop=not use_bias)
                    if use_bias:
                        nc.tensor.matmul(pse[:, 0:256], ones[:], bd[:], start=False, stop=True)
                    nc.scalar.activation(er_all[:, t, :], pse[:, 0:256],
                                         mybir.ActivationFunctionType.Copy)

            # ---- phase A1 (cont): hi-table groups ----
            for g in range(ng_lo, n_groups):
                a1_group(g)

            # ---- phase B ----
            for t in range(T):
                tidx = idxp.tile([128, 128], I16, tag="tidx")
                with tc.high_priority():
                    nc.sync.dma_start(tidx[:], gidx_d.ap()[t])
                tglo = gat.tile([128, LO_CHUNKS, 768], FP8, tag="tglo")
                nc.gpsimd.dma_gather(tglo[:], tab_lo.ap()[:],
                                     tidx[:, 0:64], LO_CAP, LO_CAP, 768,
                                     queue_num=(2 * t) % 4)
                tm = mkp.tile([128, NCHUNK, 128], FP16, tag="tm")
                tmT = mkp.tile([128, NCHUNK, 128], FP16, tag="tmT")
                with tc.high_priority():
                    nc.scalar.dma_start(tm[:], mk_d.ap()[t])
                    nc.scalar.dma_start(tmT[:], mkT_d.ap()[t])

                pagg = psA.tile([128, 260], FP32, tag="pagg")
                tghi = None
                for gi in range(NG):
                    if gi == LO_CHUNKS // sgrp:
                        # hi-table gather issued mid-tile: lo-chunk compute
                        # overlaps it (and, for early tiles, the hi-table
                        # projection tail)
                        tghi = gat.tile([128, NCHUNK - LO_CHUNKS, 768], FP8,
                                        tag="tghi")
                        nc.gpsimd.dma_gather(tghi[:], tab_hi.ap()[:],
                                             tidx[:, 64:128], HI_CAP, HI_CAP, 768,
                                             queue_num=(2 * t + 1) % 4)
                    psu = psU.tile([128, sgrp, 256], FP32, tag="psu")
                    for j in range(sgrp):
                        c = gi * sgrp + j
                        nc.tensor.matmul(psu[:, j, :], tmT[:, c, :], er_all[:, t, :],
                                         start=True, stop=False)
                        tgc = tglo[:, c, 0:256] if c < LO_CHUNKS else tghi[:, c - LO_CHUNKS, 0:256]
                        nc.tensor.matmul(psu[:, j, :], ident[:], tgc,
                                         start=False, stop=True)
                    lrl = lwp.tile([128, sgrp * 256], FP16, tag="lrl")
                    nc.scalar.activation(lrl[:], psu[:].rearrange("p a b -> p (a b)"),
                                         mybir.ActivationFunctionType.Prelu, alpha=alpha[:])
                    lsg = lwp.tile([128, sgrp * 256], FP16, tag="lsg")
                    nc.vector.tensor_tensor(lsg[:], lrl[:], sig[:, 0:sgrp * 256],
                                            op=mybir.AluOpType.mult)
                    red = smp.tile([128, sgrp * 4], FP32, tag="red")
                    nc.vector.tensor_reduce(red[:], lsg[:].rearrange("p (a h d) -> p (a h) d", h=4, d=64),
                                            axis=mybir.AxisListType.X,
                                            op=mybir.AluOpType.add)
                    msgs = msp.tile([128, sgrp, 260], FP16, tag="msgs")
                    nc.scalar.activation(msgs[:, :, 256:260],
                                         red[:].rearrange("p (a h) -> p a h", h=4),
                                         mybir.ActivationFunctionType.Exp)
                    nc.vector.tensor_tensor(
                        msgs[:, :, 0:256].rearrange("p a (h d) -> p a h d", h=4, d=64),
                        (tglo if gi * sgrp < LO_CHUNKS else tghi)[:,
                            gi * sgrp - (0 if gi * sgrp < LO_CHUNKS else LO_CHUNKS):
                            gi * sgrp - (0 if gi * sgrp < LO_CHUNKS else LO_CHUNKS) + sgrp,
                            256:768].bitcast(FP16).rearrange("p a (h d) -> p a h d", h=4, d=64),
                        msgs[:, :, 256:260].unsqueeze(-1).broadcast_to([128, sgrp, 4, 64]),
                        op=mybir.AluOpType.mult)
                    for j in range(sgrp):
                        c = gi * sgrp + j
                        nc.tensor.matmul(pagg[:], tm[:, c, :], msgs[:, j, :],
                                         start=(c == 0), stop=(c == NCHUNK - 1))

                rec = smp.tile([128, 4], FP32, tag="rec")
                nc.vector.tensor_scalar(rec[:], pagg[:, 256:260], 1e-30, None,
                                        op0=mybir.AluOpType.add)
                nc.vector.reciprocal(rec[:], rec[:])
                outt = outp.tile([128, 256], FP32, tag="outt")
                nc.vector.tensor_tensor(
                    outt[:].rearrange("p (h d) -> p h d", h=4),
                    pagg[:, 0:256].rearrange("p (h d) -> p h d", h=4),
                    rec[:].unsqueeze(-1).broadcast_to([128, 4, 64]),
                    op=mybir.AluOpType.mult)
                nc.sync.dma_start(out_d.ap()[t], outt[:])

    nc.compile()
    return nc


# ---------------------------------------------------------------------------
# kernel entry point
# ---------------------------------------------------------------------------

TRACE = False
LAST_RESULTS = None


def _ntff_hook_shim():
    """Register the axon NTFF profile hook if the antenv shim is missing."""
    import types
    try:
        from antenv.axon_hooks import get_axon_ntff_profile_hook  # noqa: F401
        return
    except ImportError:
        pass
    try:
        if '/root/.axon_site' not in sys.path:
            sys.path.insert(0, '/root/.axon_site')
        from trn_agent_boot.trn_boot import _ntff_profile_via_ctypes
        hook = _ntff_profile_via_ctypes('/opt/axon/libaxon_pjrt.so')
        mod = types.ModuleType('antenv.axon_hooks')
        mod.get_axon_ntff_profile_hook = lambda: hook
        sys.modules['antenv.axon_hooks'] = mod
    except Exception:
        pass


def _make_in_maps(meta, shared, per_core):
    T, PN = meta["T"], meta["PN"]
    ident = np.eye(128, dtype=ml_dtypes.float8_e4m3)
    maps = []
    for pc in per_core:
        maps.append(dict(
            featT=shared["featT16"].reshape(2, 128, PN),
            fownT=pc["fownT16"].reshape(2, 128, T * 128),
            wsv=shared["WsWv"], wd=shared["WdT16"],
            bsv=shared["bias_sv"], bd=shared["bias_d"],
            sig=shared["sigma_rep"],
            gidx=pc["gidx"], ident=ident,
            mk=pc["masks"].astype(np.float16),
            mkT=pc["maskT"].astype(np.float16),
        ))
    return maps


def kernel(feat, Wsrc, bsrc, Wdst, bdst, Wv, bv, attn, src, dst):
    global LAST_RESULTS
    from concourse.bass_utils import run_bass_kernel_spmd

    feat = np.asarray(feat, dtype=np.float32)
    args = dict(feat=feat,
                Wsrc=np.asarray(Wsrc, np.float32), bsrc=np.asarray(bsrc, np.float32),
                Wdst=np.asarray(Wdst, np.float32), bdst=np.asarray(bdst, np.float32),
                Wv=np.asarray(Wv, np.float32), bv=np.asarray(bv, np.float32),
                attn=np.asarray(attn, np.float32),
                src=np.asarray(src), dst=np.asarray(dst))
    N = feat.shape[0]

    meta, shared, per_core = preprocess(**args)
    nc = build_program(meta, mask_fp8=False)
    in_maps = _make_in_maps(meta, shared, per_core)

    kwargs = {}
    if TRACE:
        _ntff_hook_shim()
        kwargs["trace"] = True
    res = run_bass_kernel_spmd(nc, in_maps, core_ids=list(range(NCORES)), **kwargs)
    LAST_RESULTS = res

    out = np.zeros((N, HD), np.float32)
    nb = meta["node_b"]
    for c, r in enumerate(res.results):
        staged = r["out"].reshape(meta["T"] * 128, HD)
        oix = per_core[c]["outidx"].reshape(-1)
        valid = oix < meta["NL"]
        out[nb[c] + oix[valid]] = staged[valid]
    return out.reshape(N, H, D)



# revision 29
# speedup vs baseline: 1.1197x; 1.0107x over previous
"""DRAGConv (GATv2-style edge-softmax message passing) on 8 trn2 cores.

Strategy (dst-sorted, edge-gather, mask-matmul aggregation):
  - Host: fold |attn| into Wsrc/Wdst (leaky-relu sign trick), sort edges by
    dst, partition nodes into 8 contiguous ranges with ~equal edge counts,
    pack per-core tiles (<=128 dst nodes, <=1024 lo-src edges, <=1024
    hi-src edges), build per-tile gather indices + one-hot masks.
  - Device phase A: project el'|v tables for ALL nodes (replicated), er'
    for own nodes only (kept in SBUF).
  - Device phase B: per tile: dma_gather [el'|v] rows for 2048 edge slots,
    per 128-slot chunk: psum_u = MT_c @ er_tile + I @ el_rows;
    L = LeakyRelu(psum_u); logits = reduce(L * sigma) per head;
    ex = exp(logits); msgs = [v * ex | ex]; psum_agg += M_c @ msgs.
    Tail: out = psum_agg[:, :256] / denom per head, indirect-scatter to
    the core-local output rows.

Math: leaky_relu(x, a) . attn == sum_d sigma_d * leaky_relu(u_d, a) where
u = x * |attn| (fold into W) and sigma = sign(attn), because
LR(x)*w = sign(w) * LR(x*|w|).
"""
import sys

try:  # the runtime image ships concourse on the nix path
    import concourse.bass  # noqa: F401
except ImportError:  # fall back to the repo checkout
    sys.path.insert(0, "/opt/trn_rl_repo")

import numpy as np
import ml_dtypes

H, D = 4, 64
HD = H * D
ALPHA = 0.2
NCORES = 8
SLOTS = 2048          # edge slots per tile (16 chunks x 128)
NCHUNK = 16
LO_CHUNKS = 8         # chunks 0..7 from lo table, 8..15 from hi table
LO_CAP = LO_CHUNKS * 128
HI_CAP = (NCHUNK - LO_CHUNKS) * 128
TILE_NODES = 128


def preprocess(feat, Wsrc, bsrc, Wdst, bdst, Wv, bv, attn, src, dst):
    N, IN = feat.shape
    E = src.shape[0]
    src = np.asarray(src).astype(np.int64)
    dst = np.asarray(dst).astype(np.int64)

    attn_f = np.asarray(attn).reshape(-1).astype(np.float32)
    # keep el'/er' at natural scale (fp8-friendly); apply the full signed
    # attn weight in the post-LeakyReLU multiply instead (LR is positively
    # homogeneous, so |a|*LR(x) == LR(|a|*x) -- but the former keeps the
    # gathered el table out of fp8 subnormal range)
    sigma = attn_f
    Wsrc_s = np.asarray(Wsrc).astype(np.float32)
    Wdst_s = np.asarray(Wdst).astype(np.float32)
    bsrc_s = np.asarray(bsrc).astype(np.float32)
    bdst_s = np.asarray(bdst).astype(np.float32)
    Wv_f = np.asarray(Wv).astype(np.float32)
    bv_f = np.asarray(bv).astype(np.float32)

    use_bias = bool(np.abs(bsrc_s).max() > 0 or np.abs(bdst_s).max() > 0
                    or np.abs(bv_f).max() > 0)

    # node-tile padding for the projection table (lo/hi split at 196*128)
    PN = ((N + 127) // 128) * 128
    NSPLIT = (PN // 2 // 128) * 128  # multiple of 128, < 32768
    assert NSPLIT < 32768 and PN - NSPLIT < 32768

    # dst-sorted edges
    order = np.argsort(dst, kind="stable")
    src_s = src[order]
    dst_s = dst[order]

    # core boundaries: contiguous node ranges with ~equal edge counts
    deg = np.bincount(dst_s, minlength=N)
    cume = np.concatenate([[0], np.cumsum(deg)])  # cume[n] = edges before node n
    node_b = [0]
    for c in range(1, NCORES):
        node_b.append(int(np.searchsorted(cume, c * E / NCORES)))
    node_b.append(N)

    # per-core tile packing (vectorized: prefix sums of lo/hi degree)
    lo_deg = np.bincount(dst_s[src_s < NSPLIT], minlength=N)
    cum_lo = np.concatenate([[0], np.cumsum(lo_deg)])
    cum_hi = cume - cum_lo
    cores = []
    for c in range(NCORES):
        nlo_n, nhi_n = node_b[c], node_b[c + 1]
        tiles = []  # each: (node_start, node_count, edge_start, edge_count)
        n = nlo_n
        while n < nhi_n:
            n0 = n
            n = min(
                n0 + TILE_NODES, nhi_n,
                int(np.searchsorted(cum_lo, cum_lo[n0] + LO_CAP, side="right")) - 1,
                int(np.searchsorted(cum_hi, cum_hi[n0] + HI_CAP, side="right")) - 1,
            )
            assert n > n0, f"node {n0} degree exceeds tile caps"
            tiles.append((n0, n - n0, int(cume[n0]), int(cume[n] - cume[n0])))
        cores.append(tiles)

    T = max(len(t) for t in cores)
    NL = max(node_b[c + 1] - node_b[c] for c in range(NCORES))
    NL = ((NL + 127) // 128) * 128

    # per-core data arrays
    f8 = ml_dtypes.float8_e4m3
    per_core = []
    for c in range(NCORES):
        tiles = cores[c]
        gidx = np.zeros((T, 128, 128), np.int16)     # dma_gather index layout (16-wrap tiled 8x)
        masks = np.zeros((T, 128, NCHUNK, 128), f8)   # lhsT for aggregate: [slot][chunk][node]
        maskT = np.zeros((T, 128, NCHUNK, 128), f8)   # lhsT for expand: [node][chunk][slot]
        outidx = np.full((T, 128, 1), NL, np.int32)   # local out row per node slot (NL => dropped)
        ernode = np.zeros((T * 128,), np.int64)       # global node per er slot
        ervalid = np.zeros((T * 128,), bool)

        for t, (n0, nn, e0, ne) in enumerate(tiles):
            es = src_s[e0:e0 + ne]
            ed = dst_s[e0:e0 + ne]
            lo_m = es < NSPLIT
            lo_src = es[lo_m]
            hi_src = es[~lo_m] - NSPLIT
            lsl = np.arange(lo_src.shape[0])          # slots 0..
            hsl = HI_CAP * 0 + LO_CAP + np.arange(hi_src.shape[0])
            slot = np.empty(ne, np.int64)
            slot[lo_m] = lsl
            slot[~lo_m] = hsl
            local_node = ed - n0                      # 0..nn-1

            # gather indices (seq position i lives at [i%16, i//16] of [16,128])
            seq = np.zeros(SLOTS, np.int16)
            seq[lsl] = lo_src.astype(np.int16)
            seq[LO_CAP + np.arange(hi_src.shape[0])] = hi_src.astype(np.int16)
            gidx[t] = np.tile(seq.reshape(16, 128, order="F"), (8, 1))  # [i%16, i//16], replicated per Q7 core

            ch = slot // 128
            sl = slot % 128
            masks[t, sl, ch, local_node] = 1.0
            maskT[t, local_node, ch, sl] = 1.0
            outidx[t, :nn, 0] = (n0 - node_b[c]) + np.arange(nn)
            ernode[t * 128: t * 128 + nn] = n0 + np.arange(nn)
            ervalid[t * 128: t * 128 + nn] = True

        # er projection source: feat rows in tile-slot order, transposed fp16
        fown = np.zeros((T * 128, IN), np.float32)
        fown[ervalid] = feat[ernode[ervalid]]
        fownT16 = np.ascontiguousarray(fown.T).astype(np.float16)

        per_core.append(dict(gidx=gidx, masks=masks, maskT=maskT,
                             outidx=outidx, fownT16=fownT16))

    featP = np.zeros((PN, IN), np.float32)
    featP[:N] = feat
    featT16 = np.ascontiguousarray(featP.T).astype(np.float16)

    WsWv = np.concatenate([Wsrc_s.T, Wv_f.T], axis=1).astype(np.float16)  # [IN, 512]
    WdT16 = np.ascontiguousarray(Wdst_s.T).astype(np.float16)             # [IN, 256]
    bias_sv = np.concatenate([bsrc_s, bv_f]).reshape(1, 512).astype(np.float16)
    bias_d = bdst_s.reshape(1, 256).astype(np.float16)
    sigma_rep = np.tile(np.repeat(sigma.reshape(1, HD), 128, axis=0).astype(np.float16), (1, 4))

    meta = dict(N=N, E=E, IN=IN, PN=PN, NSPLIT=NSPLIT, T=T, NL=NL,
                node_b=node_b, use_bias=use_bias)
    shared = dict(featT16=featT16, WsWv=WsWv, WdT16=WdT16,
                  bias_sv=bias_sv, bias_d=bias_d, sigma_rep=sigma_rep)
    return meta, shared, per_core


# ---------------------------------------------------------------------------
# device program
# ---------------------------------------------------------------------------

def build_program(meta, mask_fp8=False, sgrp=4):
    import concourse.bass as bass
    import concourse.tile as tile
    from concourse import bacc, mybir

    FP16 = mybir.dt.float16
    FP32 = mybir.dt.float32
    FP8 = mybir.dt.float8e4
    I16 = mybir.dt.int16
    I32 = mybir.dt.int32
    ts = bass.ts

    IN = meta["IN"]
    PN, NSPLIT, T, NL = meta["PN"], meta["NSPLIT"], meta["T"], meta["NL"]
    use_bias = meta["use_bias"]
    PT = PN // 128
    PT_LO = NSPLIT // 128
    HI_ROWS = PN - NSPLIT
    MDT = FP8 if mask_fp8 else FP16
    NG = NCHUNK // sgrp  # chunk groups per tile

    nc = bacc.Bacc("TRN2", target_bir_lowering=False, debug=False, num_devices=1,
                   num_swdge_queues=4)

    featT = nc.dram_tensor("featT", [2, 128, PN], FP16, kind="ExternalInput")
    fownT = nc.dram_tensor("fownT", [2, 128, T * 128], FP16, kind="ExternalInput")
    wsv_d = nc.dram_tensor("wsv", [IN, 512], FP16, kind="ExternalInput")
    wd_d = nc.dram_tensor("wd", [IN, 256], FP16, kind="ExternalInput")
    bsv_d = nc.dram_tensor("bsv", [1, 512], FP16, kind="ExternalInput")
    bd_d = nc.dram_tensor("bd", [1, 256], FP16, kind="ExternalInput")
    sig_d = nc.dram_tensor("sig", [128, 1024], FP16, kind="ExternalInput")
    gidx_d = nc.dram_tensor("gidx", [T, 128, 128], I16, kind="ExternalInput")
    mk_d = nc.dram_tensor("mk", [T, 128, NCHUNK, 128], MDT, kind="ExternalInput")
    mkT_d = nc.dram_tensor("mkT", [T, 128, NCHUNK, 128], MDT, kind="ExternalInput")
    ident_d = nc.dram_tensor("ident", [128, 128], FP8, kind="ExternalInput")
    out_d = nc.dram_tensor("out", [T, 128, 256], FP32, kind="ExternalOutput")

    # table row layout: 256 x fp8 el' | 512 bytes (256 x fp16, bitcast) v.
    # fp8 el costs ~3% relative noise on the pre-activation attention input
    # (well inside tolerance) and cuts table write + gather bytes by 25%.
    tab_lo = nc.dram_tensor("tab_lo", [NSPLIT, 768], FP8, kind="Internal")
    tab_hi = nc.dram_tensor("tab_hi", [HI_ROWS, 768], FP8, kind="Internal")

    with tile.TileContext(nc) as tc:
        with (
            tc.tile_pool(name="const", bufs=1) as constp,
            tc.tile_pool(name="erall", bufs=1) as erp,
            tc.tile_pool(name="ftin", bufs=3) as ftp,
            tc.tile_pool(name="gout", bufs=3) as gop,
            tc.tile_pool(name="psU", bufs=3, space="PSUM") as psU,
            tc.tile_pool(name="psA", bufs=2, space="PSUM") as psA,
            tc.tile_pool(name="gath", bufs=4) as gat,
            tc.tile_pool(name="mask", bufs=4) as mkp,
            tc.tile_pool(name="idxp", bufs=2) as idxp,
            tc.tile_pool(name="lwork", bufs=4) as lwp,
            tc.tile_pool(name="msgs", bufs=4) as msp,
            tc.tile_pool(name="small", bufs=4) as smp,
            tc.tile_pool(name="outp", bufs=2) as outp,
        ):
            # constants
            wsv0 = constp.tile([128, 512], FP16, tag="wsv0")
            nc.sync.dma_start(wsv0[:], wsv_d.ap()[0:128, :])
            wsv1 = constp.tile([128, 512], FP16, tag="wsv1")
            nc.sync.dma_start(wsv1[:], wsv_d.ap()[128:256, :])
            wd0 = constp.tile([128, 256], FP16, tag="wd0")
            nc.sync.dma_start(wd0[:], wd_d.ap()[0:128, :])
            wd1 = constp.tile([128, 256], FP16, tag="wd1")
            nc.sync.dma_start(wd1[:], wd_d.ap()[128:256, :])
            sig = constp.tile([128, 1024], FP16, tag="sig")
            nc.sync.dma_start(sig[:], sig_d.ap()[:])
            ident = constp.tile([128, 128], FP8, tag="ident")
            nc.sync.dma_start(ident[:], ident_d.ap()[:])
            alpha = constp.tile([128, 1], FP32, tag="alpha")
            nc.vector.memset(alpha[:], ALPHA)
            if use_bias:
                bsv = constp.tile([1, 512], FP16, tag="bsv")
                nc.sync.dma_start(bsv[:], bsv_d.ap()[:])
                bd = constp.tile([1, 256], FP16, tag="bd")
                nc.sync.dma_start(bd[:], bd_d.ap()[:])
                ones = constp.tile([1, 128], FP16, tag="ones")
                nc.vector.memset(ones[:], 1.0)

            er_all = erp.tile([128, T, 256], FP16, tag="er_all")

            GRP = 8  # node tiles per featT load group

            # ---- phase A1: el'|v tables for all nodes (replicated) ----
            # Emitted as lo-table groups, then er' (A2), then hi-table groups:
            # tab_lo + er_all complete early so phase-B lo gathers and compute
            # overlap the hi-table tail.
            def a1_group(g):
                npt = min(GRP, PT - g * GRP)
                w = npt * 128
                f0 = ftp.tile([128, GRP * 128], FP16, tag="f0")
                nc.scalar.dma_start(f0[:, 0:w], featT.ap()[0, :, g * GRP * 128:g * GRP * 128 + w])
                f1 = ftp.tile([128, GRP * 128], FP16, tag="f1")
                nc.scalar.dma_start(f1[:, 0:w], featT.ap()[1, :, g * GRP * 128:g * GRP * 128 + w])
                g16 = gop.tile([128, GRP, 768], FP8, tag="g16")
                for j in range(npt):
                    psgt = psU.tile([128, sgrp, 256], FP32, tag="psu")
                    psg = psgt.rearrange("p a b -> p (a b)")[:, 0:512]
                    nc.tensor.matmul(psg, f0[:, ts(j, 128)], wsv0[:], start=True, stop=False)
                    nc.tensor.matmul(psg, f1[:, ts(j, 128)], wsv1[:],
                                     start=False, stop=not use_bias)
                    if use_bias:
                        nc.tensor.matmul(psg, ones[:], bsv[:], start=False, stop=True)
                    # PSUM casts (el -> fp8, v -> fp16 via bitcast view):
                    # alternate engines so neither becomes the phase-A pacer
                    eng = nc.vector if j % 2 == 0 else nc.scalar
                    if j % 2 == 0:
                        eng.tensor_copy(g16[:, j, 0:256], psg[:, 0:256])
                        eng.tensor_copy(g16[:, j, 256:768].bitcast(FP16),
                                        psg[:, 256:512])
                    else:
                        eng.activation(g16[:, j, 0:256], psg[:, 0:256],
                                       mybir.ActivationFunctionType.Copy)
                        eng.activation(g16[:, j, 256:768].bitcast(FP16),
                                       psg[:, 256:512],
                                       mybir.ActivationFunctionType.Copy)
                # one batched write per group; rows pt*128..(pt+npt)*128 of the
                # combined table, split at the lo/hi boundary if it lands inside
                p0 = g * GRP
                segs = []
                if p0 < PT_LO:
                    e = min(p0 + npt, PT_LO)
                    segs.append((0, e - p0, tab_lo, p0))
                if p0 + npt > PT_LO:
                    s = max(p0, PT_LO)
                    segs.append((s - p0, p0 + npt - s, tab_hi, s - PT_LO))
                for (joff, cnt, tab, rt) in segs:
                    nc.sync.dma_start(
                        tab.ap()[rt * 128:(rt + cnt) * 128, :].rearrange(
                            "(a p) w -> p a w", p=128),
                        g16[:, joff:joff + cnt, :])

            n_groups = (PT + GRP - 1) // GRP
            ng_lo = min((PT_LO + GRP - 1) // GRP, n_groups)
            for g in range(ng_lo):
                a1_group(g)

            # ---- phase A2: er' for own nodes, kept in SBUF ----
            for g in range((T + GRP - 1) // GRP):
                npt = min(GRP, T - g * GRP)
                w = npt * 128
                f0 = ftp.tile([128, GRP * 128], FP16, tag="f0")
                nc.scalar.dma_start(f0[:, 0:w], fownT.ap()[0, :, g * GRP * 128:g * GRP * 128 + w])
                f1 = ftp.tile([128, GRP * 128], FP16, tag="f1")
                nc.scalar.dma_start(f1[:, 0:w], fownT.ap()[1, :, g * GRP * 128:g * GRP * 128 + w])
                for j in range(npt):
                    t = g * GRP + j
                    pset = psU.tile([128, sgrp, 256], FP32, tag="psu")
                    pse = pset.rearrange("p a b -> p (a b)")
                    nc.tensor.matmul(pse[:, 0:256], f0[:, ts(j, 128)], wd0[:], start=True, stop=False)
                    nc.tensor.matmul(pse[:, 0:256], f1[:, ts(j, 128)], wd1[:],
                                     start=False, stop=not use_bias)
                    if use_bias:
                        nc.tensor.matmul(pse[:, 0:256], ones[:], bd[:], start=False, stop=True)
                    nc.scalar.activation(er_all[:, t, :], pse[:, 0:256],
                                         mybir.ActivationFunctionType.Copy)

            # ---- phase A1 (cont): hi-table groups ----
            for g in range(ng_lo, n_groups):
                a1_group(g)

            # ---- phase B ----
            for t in range(T):
                tidx = idxp.tile([128, 128], I16, tag="tidx")
                with tc.high_priority():
                    nc.sync.dma_start(tidx[:], gidx_d.ap()[t])
                tglo = gat.tile([128, LO_CHUNKS, 768], FP8, tag="tglo")
                nc.gpsimd.dma_gather(tglo[:], tab_lo.ap()[:],
                                     tidx[:, 0:64], LO_CAP, LO_CAP, 768,
                                     queue_num=(2 * t) % 4)
                tm = mkp.tile([128, NCHUNK, 128], FP16, tag="tm")
                tmT = mkp.tile([128, NCHUNK, 128], FP16, tag="tmT")
                with tc.high_priority():
                    nc.scalar.dma_start(tm[:], mk_d.ap()[t])
                    nc.scalar.dma_start(tmT[:], mkT_d.ap()[t])

                pagg = psA.tile([128, 260], FP32, tag="pagg")
                tghi = None
                for gi in range(NG):
                    if gi == LO_CHUNKS // sgrp:
                        # hi-table gather issued mid-tile: lo-chunk compute
                        # overlaps it (and, for early tiles, the hi-table
                        # projection tail)
                        tghi = gat.tile([128, NCHUNK - LO_CHUNKS, 768], FP8,
                                        tag="tghi")
                        nc.gpsimd.dma_gather(tghi[:], tab_hi.ap()[:],
                                             tidx[:, 64:128], HI_CAP, HI_CAP, 768,
                                             queue_num=(2 * t + 1) % 4)
                    psu = psU.tile([128, sgrp, 256], FP32, tag="psu")
                    for j in range(sgrp):
                        c = gi * sgrp + j
                        nc.tensor.matmul(psu[:, j, :], tmT[:, c, :], er_all[:, t, :],
                                         start=True, stop=False)
                        tgc = tglo[:, c, 0:256] if c < LO_CHUNKS else tghi[:, c - LO_CHUNKS, 0:256]
                        nc.tensor.matmul(psu[:, j, :], ident[:], tgc,
                                         start=False, stop=True)
                    lrl = lwp.tile([128, sgrp * 256], FP16, tag="lrl")
                    nc.scalar.activation(lrl[:], psu[:].rearrange("p a b -> p (a b)"),
                                         mybir.ActivationFunctionType.Prelu, alpha=alpha[:])
                    lsg = lwp.tile([128, sgrp * 256], FP16, tag="lsg")
                    nc.vector.tensor_tensor(lsg[:], lrl[:], sig[:, 0:sgrp * 256],
                                            op=mybir.AluOpType.mult)
                    red = smp.tile([128, sgrp * 4], FP32, tag="red")
                    nc.vector.tensor_reduce(red[:], lsg[:].rearrange("p (a h d) -> p (a h) d", h=4, d=64),
                                            axis=mybir.AxisListType.X,
                                            op=mybir.AluOpType.add)
                    msgs = msp.tile([128, sgrp, 260], FP16, tag="msgs")
                    nc.scalar.activation(msgs[:, :, 256:260],
                                         red[:].rearrange("p (a h) -> p a h", h=4),
                                         mybir.ActivationFunctionType.Exp)
                    nc.vector.tensor_tensor(
                        msgs[:, :, 0:256].rearrange("p a (h d) -> p a h d", h=4, d=64),
                        (tglo if gi * sgrp < LO_CHUNKS else tghi)[:,
                            gi * sgrp - (0 if gi * sgrp < LO_CHUNKS else LO_CHUNKS):
                            gi * sgrp - (0 if gi * sgrp < LO_CHUNKS else LO_CHUNKS) + sgrp,
                            256:768].bitcast(FP16).rearrange("p a (h d) -> p a h d", h=4, d=64),
                        msgs[:, :, 256:260].unsqueeze(-1).broadcast_to([128, sgrp, 4, 64]),
                        op=mybir.AluOpType.mult)
                    for j in range(sgrp):
                        c = gi * sgrp + j
                        nc.tensor.matmul(pagg[:], tm[:, c, :], msgs[:, j, :],
                                         start=(c == 0), stop=(c == NCHUNK - 1))

                rec = smp.tile([128, 4], FP32, tag="rec")
                nc.vector.tensor_scalar(rec[:], pagg[:, 256:260], 1e-30, None,
                                        op0=mybir.AluOpType.add)
                nc.vector.reciprocal(rec[:], rec[:])
                outt = outp.tile([128, 256], FP32, tag="outt")
                nc.vector.tensor_tensor(
                    outt[:].rearrange("p (h d) -> p h d", h=4),
                    pagg[:, 0:256].rearrange("p (h d) -> p h d", h=4),
                    rec[:].unsqueeze(-1).broadcast_to([128, 4, 64]),
                    op=mybir.AluOpType.mult)
                nc.sync.dma_start(out_d.ap()[t], outt[:])

    nc.compile()
    return nc


# ---------------------------------------------------------------------------
# kernel entry point
# ---------------------------------------------------------------------------

TRACE = False
LAST_RESULTS = None


def _ntff_hook_shim():
    """Register the axon NTFF profile hook if the antenv shim is missing."""
    import types
    try:
        from antenv.axon_hooks import get_axon_ntff_profile_hook  # noqa: F401
        return
    except ImportError:
        pass
    try:
        if '/root/.axon_site' not in sys.path:
            sys.path.insert(0, '/root/.axon_site')
        from trn_agent_boot.trn_boot import _ntff_profile_via_ctypes
        hook = _ntff_profile_via_ctypes('/opt/axon/libaxon_pjrt.so')
        mod = types.ModuleType('antenv.axon_hooks')
        mod.get_axon_ntff_profile_hook = lambda: hook
        sys.modules['antenv.axon_hooks'] = mod
    except Exception:
        pass


def _make_in_maps(meta, shared, per_core):
    T, PN = meta["T"], meta["PN"]
    ident = np.eye(128, dtype=ml_dtypes.float8_e4m3)
    maps = []
    for pc in per_core:
        maps.append(dict(
            featT=shared["featT16"].reshape(2, 128, PN),
            fownT=pc["fownT16"].reshape(2, 128, T * 128),
            wsv=shared["WsWv"], wd=shared["WdT16"],
            bsv=shared["bias_sv"], bd=shared["bias_d"],
            sig=shared["sigma_rep"],
            gidx=pc["gidx"], ident=ident,
            mk=pc["masks"].astype(np.float16),
            mkT=pc["maskT"].astype(np.float16),
        ))
    return maps


def kernel(feat, Wsrc, bsrc, Wdst, bdst, Wv, bv, attn, src, dst):
    global LAST_RESULTS
    from concourse.bass_utils import run_bass_kernel_spmd

    feat = np.asarray(feat, dtype=np.float32)
    args = dict(feat=feat,
                Wsrc=np.asarray(Wsrc, np.float32), bsrc=np.asarray(bsrc, np.float32),
                Wdst=np.asarray(Wdst, np.float32), bdst=np.asarray(bdst, np.float32),
                Wv=np.asarray(Wv, np.float32), bv=np.asarray(bv, np.float32),
                attn=np.asarray(attn, np.float32),
                src=np.asarray(src), dst=np.asarray(dst))
    N = feat.shape[0]

    meta, shared, per_core = preprocess(**args)
    nc = build_program(meta, mask_fp8=False)
    in_maps = _make_in_maps(meta, shared, per_core)

    kwargs = {}
    if TRACE:
        _ntff_hook_shim()
        kwargs["trace"] = True
    res = run_bass_kernel_spmd(nc, in_maps, core_ids=list(range(NCORES)), **kwargs)
    LAST_RESULTS = res

    out = np.zeros((N, HD), np.float32)
    nb = meta["node_b"]
    for c, r in enumerate(res.results):
        staged = r["out"].reshape(meta["T"] * 128, HD)
        oix = per_core[c]["outidx"].reshape(-1)
        valid = oix < meta["NL"]
        out[nb[c] + oix[valid]] = staged[valid]
    return out.reshape(N, H, D)

